# revision 32
# baseline (speedup 1.0000x reference)
"""Trainium2 Bass kernel for the MFA/MPPCA mixture log-likelihood problem.

Math: out[n,k] = PI[k] + logprob[n,k] with Sigma_k = A_k A_k^T + diag(D_k^2),
computed via Woodbury.  Everything involving only the small parameters
(MU, A, D, PI) is folded on the host into:

    out[n,k] = CONST[k] + x[n]·H[:,k] + (x[n]^2)·G[:,k] + sum_l (x[n]·Csc[:,k,l])^2

where (with iD = D^-2, B = iD*A, L = I + A^T B, iL = inv(L), R = chol(iL),
C0 = B R, e = R^T B^T MU):
    G   = -0.5 * iD^T                       (d, K)
    H   = (iD*MU)^T - C0 e                  (d, K)
    Csc = sqrt(0.5) * C0                    (d, K*l)
    CONST = PI - 0.5*(d log 2pi + logdet Sigma + MU^T iD MU) + 0.5 |e|^2

Device kernel (data-parallel over N on 8 cores), all matmuls fp8e4 with
DoubleRow perf mode (2 fp8 weights/cell, 256-deep contraction per matmul):

  - Csc path: x8 chunk-pairs stationary, interleaved Csc columns moving;
    640 projections per sample land in two PSUM banks; ScalarE squares
    them (one 4D-AP activation), VectorE group-sums 10 -> 1 into the
    fp16 out1 tile.
  - H and G paths both accumulate into a [k, n] PSUM tile (h8/g8
    chunk-pairs stationary - cheap 64-col weight loads - with x8 / x2h
    as the 512-wide moving streams).  ScalarE copies it out with
    CONST[k] folded in as a per-partition bias -> fp16 out2.
  - x^2 is sent from the host as fp8 (x2h); the exact-residual term was
    measured to cost more than its accuracy is worth at rel-tol 2e-2.
  - The host sums out1[n,k] + out2[k,n]^T (cheap numpy) - this removes
    the PE transpose and the vector final-add from the critical path.
"""
import math
import numpy as np
import ml_dtypes

N_TOTAL, K, D_FEAT, L_FAC = 131072, 64, 512, 10
N_CORES = 8
N_PER_CORE = N_TOTAL // N_CORES  # 16384

CSC_COLS = K * L_FAC             # 640
NG_HALF = K // 2                 # 32 groups per psum bank
CSC_HALF = NG_HALF * L_FAC       # 320
GROUP = 1024                     # samples per DMA group (8 subtiles)

FP8 = ml_dtypes.float8_e4m3      # TRN fp8e4 (max +-240)


def host_prep(MU, A, D, PI):
    """Fold small-parameter math into matmul weights (float64 internally)."""
    MU64, A64, D64, PI64 = [np.asarray(v, np.float64) for v in (MU, A, D, PI)]
    Kc, d, l = A64.shape
    iD = D64 ** -2.0
    B = iD[..., None] * A64
    L = np.eye(l)[None] + np.einsum('kdl,kdm->klm', A64, B)
    sign, logdet_L = np.linalg.slogdet(L)
    log_det_Sigma = logdet_L - np.sum(np.log(iD), axis=1)
    iL = np.linalg.inv(L)
    R = np.linalg.cholesky(iL)                  # R @ R.T = iL
    C0 = np.einsum('kdl,klm->kdm', B, R)        # (K, d, l)
    bmu = np.einsum('kdl,kd->kl', B, MU64)
    e = np.einsum('klm,kl->km', R, bmu)         # (K, l)
    c1 = np.sum(iD * MU64 * MU64, axis=1)

    CONST = PI64 - 0.5 * (d * math.log(2.0 * math.pi) + log_det_Sigma + c1) \
        + 0.5 * np.sum(e * e, axis=1)
    G = (-0.5 * iD).T                                               # (d, K)
    H = (iD * MU64 - np.einsum('kdm,km->kd', C0, e)).T              # (d, K)
    Csc = (C0 * np.sqrt(0.5)).transpose(1, 0, 2).reshape(d, Kc * l)  # k-major

    # interleave chunk pairs: [p, pair, col, 2] so the DoubleRow moving
    # stream fetches both pair values from one 16B SBUF line
    wall_i = np.ascontiguousarray(
        Csc.astype(FP8).reshape(2, 2, 128, CSC_COLS).transpose(2, 0, 3, 1))
    hg8 = np.concatenate([H, G], axis=1).astype(FP8)                # (d, 2K)
    cwide = np.tile(CONST.astype(np.float32).reshape(K, 1),
                    (1, 512)).astype(np.float32)                    # (K, 512)
    return wall_i, hg8, cwide


def build_nc(n_per_core=N_PER_CORE):
    """Build and compile the Bass module for one core (SPMD across 8)."""
    import concourse.bacc as bacc
    import concourse.tile as tile
    import concourse.mybir as mybir

    f32 = mybir.dt.float32
    f16 = mybir.dt.float16
    f8 = mybir.dt.float8e4
    DR = mybir.MatmulPerfMode.DoubleRow
    DRSI = mybir.MatmulPerfMode.DoubleRowSwInterleave
    n_groups = n_per_core // GROUP
    n_sub_g = GROUP // 128       # 8 subtiles per DMA group
    assert n_per_core % GROUP == 0

    nc = bacc.Bacc("TRN2", target_bir_lowering=False, debug=False,
                   enable_asserts=False, num_devices=N_CORES)
    x8_dram = nc.dram_tensor("x8", (128, 2, n_per_core, 2), f8,
                             kind="ExternalInput")
    x2h_dram = nc.dram_tensor("x2h", (128, 2, n_per_core, 2), f8,
                              kind="ExternalInput")
    wall_dram = nc.dram_tensor("wall8i", (128, 2, CSC_COLS, 2), f8,
                               kind="ExternalInput")
    hg_dram = nc.dram_tensor("hg8", (D_FEAT, 2 * K), f8, kind="ExternalInput")
    cw_dram = nc.dram_tensor("cwide", (K, 512), f32, kind="ExternalInput")
    out1_dram = nc.dram_tensor("out1", (n_per_core, K), f16, kind="ExternalOutput")
    out2_dram = nc.dram_tensor("out2", (K, n_per_core), f16, kind="ExternalOutput")

    x8_v = x8_dram.ap()                                    # [128,2,n,2]
    x2h_v = x2h_dram.ap()                                  # [128,2,n,2]
    wall_v = wall_dram.ap()                                # [128,2,640,2]
    hg_v = hg_dram.ap().rearrange("(c p) m -> p c m", p=128)       # [128,4,128]

    with tile.TileContext(nc) as tc:
        with (
            tc.tile_pool(name="wpool", bufs=1) as wpool,
            tc.tile_pool(name="xpool", bufs=4) as xpool,
            tc.tile_pool(name="spool", bufs=4) as spool,
            tc.tile_pool(name="opool", bufs=3) as opool,
            tc.tile_pool(name="ppool", bufs=3, space="PSUM") as ppool,
            tc.tile_pool(name="gpool", bufs=1, space="PSUM") as gpool,
        ):
            # startup ordering: a 2-subtile head slice of x8 and the wall
            # go first so the first matmuls fire as early as possible; the
            # small weight tensors follow the group-0 bulk inputs
            x8h_sb = wpool.tile([128, 2, 256, 2], f8)
            nc.sync.dma_start(out=x8h_sb[:], in_=x8_v[:, :, 0:256, :])
            wall_sb = wpool.tile([128, 2, CSC_COLS, 2], f8)
            nc.sync.dma_start(out=wall_sb[:], in_=wall_v[:])

            def issue_in_dma(gi):
                gsl = slice(gi * GROUP, (gi + 1) * GROUP)
                x8_sb = xpool.tile([128, 2, GROUP, 2], f8, tag="x8")
                nc.sync.dma_start(out=x8_sb[:], in_=x8_v[:, :, gsl, :])
                x2h_sb = xpool.tile([128, 2, GROUP, 2], f8, tag="x2h")
                nc.sync.dma_start(out=x2h_sb[:], in_=x2h_v[:, :, gsl, :])
                return x8_sb, x2h_sb

            in_bufs = {0: issue_in_dma(0)}
            hg_sb = wpool.tile([128, 4, 2 * K], f8)
            nc.sync.dma_start(out=hg_sb[:], in_=hg_v[:])
            cw_sb = wpool.tile([K, 512], f32)
            nc.sync.dma_start(out=cw_sb[:], in_=cw_dram.ap())
            c_sb = cw_sb[:, 0:1]
            if n_groups > 1:
                in_bufs[1] = issue_in_dma(1)
            # deferred half-copies of ps_g -> out2 from the PREVIOUS group,
            # flushed early in the next group so they never park the scalar/
            # vector queues behind this group's compute
            deferred = []

            for gi in range(n_groups):
                gsl = slice(gi * GROUP, (gi + 1) * GROUP)
                # prefetch next group's inputs ahead of this group's output
                # DMAs so the sync queue never parks input transfers behind
                # an output waiting on compute
                if gi + 2 < n_groups:
                    in_bufs[gi + 2] = issue_in_dma(gi + 2)
                x8_sb, x2h_sb = in_bufs.pop(gi)

                out_t = opool.tile([128, n_sub_g, K], f16, tag="out")
                out2_t = opool.tile([K, GROUP], f16, tag="out2")
                ps_g0 = gpool.tile([K, 512], f32, tag="psg0")
                ps_g1 = gpool.tile([K, 512], f32, tag="psg1")
                halves = [ps_g0, ps_g1]

                # one H/G matmul per subtile, spread across the group so the
                # PE fills the scalar-paced slack instead of a serial tail
                def psg_mm(j):
                    h, idx = j // 4, j % 4
                    hsl = slice(h * 512, (h + 1) * 512)
                    dst = halves[h][:]
                    if idx < 2:
                        cpair, csl = idx * 2, slice(0, K)
                        rhs = x8_sb[:, idx, hsl, :] \
                            .rearrange("p t two -> p two t")
                    else:
                        cpair, csl = (idx - 2) * 2, slice(K, 2 * K)
                        rhs = x2h_sb[:, (idx - 2), hsl, :] \
                            .rearrange("p n two -> p two n")
                    nc.tensor.matmul(
                        dst, hg_sb[:, cpair:cpair + 2, csl], rhs,
                        start=(idx == 0), stop=(idx == 3), perf_mode=DR,
                        skip_group_check=True)

                psg_delay = 2 if gi == 0 else 0
                for j in range(n_sub_g):
                    nsl = slice(j * 128, (j + 1) * 128)
                    # psum: bank0 = [Csc groups 0:32], bank1 = [groups 32:64]
                    ps = ppool.tile([128, 1024], f32, tag="ps")
                    stat_sb = x8h_sb if (gi == 0 and j < 2) else x8_sb

                    def dr(dst, cpair, cols, start, stop):
                        # stationary: interleaved+reversed x pairs (SwInterleave)
                        nc.tensor.matmul(
                            dst,
                            stat_sb[:, cpair // 2, nsl, :]
                            .rearrange("p t two -> p (t two)"),
                            wall_sb[:, cpair // 2, cols, :]
                            .rearrange("p n two -> p two n"),
                            start=start, stop=stop, perf_mode=DRSI,
                            skip_group_check=True)

                    if j - psg_delay >= 0:
                        psg_mm(j - psg_delay)
                    dr(ps[:, 0:CSC_HALF], 0, slice(0, CSC_HALF), True, False)
                    dr(ps[:, 512:512 + CSC_HALF], 0,
                       slice(CSC_HALF, CSC_COLS), True, False)
                    dr(ps[:, 0:CSC_HALF], 2, slice(0, CSC_HALF), False, True)
                    dr(ps[:, 512:512 + CSC_HALF], 2,
                       slice(CSC_HALF, CSC_COLS), False, True)

                    # squares of the 640 factor projections (flat 3D AP;
                    # the reduce re-views the same bytes as [p, 64, 10])
                    sq = spool.tile([128, 2, CSC_HALF], f16, tag="sq")
                    nc.scalar.square(
                        sq[:],
                        ps.rearrange("p (b h) -> p b h", b=2)[:, :, 0:CSC_HALF])

                    # group-of-10 sums straight into the fp16 output tile
                    with nc.allow_low_precision("output is fp16 anyway"):
                        nc.vector.reduce_sum(
                            out_t[:, j, :],
                            sq.rearrange("p b (g t) -> p (b g) t", t=L_FAC),
                            axis=mybir.AxisListType.X)

                    # flush one deferred op from the previous group per
                    # subtile (keeps them off the critical path)
                    if deferred:
                        deferred.pop(0)()

                    if j == 5:
                        # ps_g half 0 complete: copy with CONST bias (scalar)
                        def copy_h0(t2=out2_t, pg=ps_g0):
                            nc.scalar.add(t2[:, 0:512], pg[:], add=c_sb[:])
                        deferred.append(copy_h0)

                for jj in range(n_sub_g - psg_delay, n_sub_g):
                    psg_mm(jj)

                # half 1 + out DMAs, deferred into the next group's subtiles
                def copy_h1(t2=out2_t, pg=ps_g1):
                    # vector does half 1: psum + CONST_wide -> fp16
                    nc.vector.tensor_add(t2[:, 512:1024], pg[:], cw_sb[:])
                deferred.append(copy_h1)

                def dma_out(ot=out_t, t2=out2_t, osl=gsl):
                    nc.sync.dma_start(
                        out=out1_dram.ap()[osl, :].rearrange(
                            "(j p) k -> p j k", p=128),
                        in_=ot[:])
                    nc.sync.dma_start(out=out2_dram.ap()[:, osl], in_=t2[:])
                deferred.append(dma_out)

            for f in deferred:
                f()

    nc.compile()
    return nc


_NC_CACHE = {}


def _get_nc(n_per_core=N_PER_CORE):
    if n_per_core not in _NC_CACHE:
        _NC_CACHE[n_per_core] = build_nc(n_per_core)
    return _NC_CACHE[n_per_core]


def _install_ntff_hook():
    """Provide the antenv.axon_hooks shim so trace=True can capture NTFFs."""
    import sys
    if "antenv.axon_hooks" in sys.modules:
        return
    import types
    import ctypes
    import contextlib

    so_path = "/opt/axon/libaxon_pjrt.so"
    lib = ctypes.CDLL(so_path)
    if not hasattr(lib, "axon_start_nrt_profile"):
        return
    lib.axon_start_nrt_profile.argtypes = [ctypes.POINTER(ctypes.c_int64), ctypes.c_size_t]
    lib.axon_start_nrt_profile.restype = ctypes.c_int64
    lib.axon_stop_nrt_profile.argtypes = [ctypes.c_char_p]
    lib.axon_stop_nrt_profile.restype = ctypes.c_int64

    @contextlib.contextmanager
    def _hook(output_dir, device_ids):
        import jax
        jax.devices()
        if device_ids:
            ids = (ctypes.c_int64 * len(device_ids))(*device_ids)
            rc = lib.axon_start_nrt_profile(ids, len(device_ids))
        else:
            rc = lib.axon_start_nrt_profile(None, 0)
        if rc != 0:
            raise RuntimeError(f"axon_start_nrt_profile rc={rc}")
        try:
            yield
        finally:
            n = lib.axon_stop_nrt_profile(str(output_dir).encode())
            print(f"ntff profile: {n} file(s) written to {output_dir}")

    mod = types.ModuleType("antenv.axon_hooks")
    mod.get_axon_ntff_profile_hook = lambda: _hook
    mod.set_axon_ntff_profile_hook = lambda h: None
    sys.modules["antenv.axon_hooks"] = mod


def kernel(x, MU, A, D, PI, trace=False):
    from concourse.bass_utils import run_bass_kernel_spmd
    if trace:
        try:
            _install_ntff_hook()
        except Exception as e:
            print(f"ntff hook install failed: {e}")
            trace = False

    x = np.asarray(x, np.float32)
    wall, hg8, cwide = host_prep(MU, A, D, PI)
    nc = _get_nc()

    def ileave_rev(a):
        # (512, n) -> (128, pair, n, 2): chunk-pair interleave with samples
        # reversed within each 128-block (DoubleRowSwInterleave weight order;
        # the H/G moving streams use the same order so psum columns agree)
        n = a.shape[1]
        return np.ascontiguousarray(
            a.reshape(2, 2, 128, n // 128, 128)[..., ::-1]
            .transpose(2, 0, 3, 4, 1).reshape(128, 2, n, 2))

    in_maps = []
    for c in range(N_CORES):
        xs = np.ascontiguousarray(x[c * N_PER_CORE:(c + 1) * N_PER_CORE, :].T)
        x8 = xs.astype(FP8)
        x2h = (xs * xs).astype(FP8)
        in_maps.append({"x8": ileave_rev(x8), "x2h": ileave_rev(x2h),
                        "wall8i": wall, "hg8": hg8, "cwide": cwide})

    res = run_bass_kernel_spmd(nc, in_maps, list(range(N_CORES)), trace=trace)
    parts = []
    for c in range(N_CORES):
        o1 = res.results[c]["out1"].astype(np.float32)
        o2 = res.results[c]["out2"].astype(np.float32)
        o2 = o2.reshape(K, -1, 128)[:, :, ::-1].reshape(K, N_PER_CORE)
        parts.append(o1 + o2.T)
    out = np.concatenate(parts, axis=0)
    if trace:
        kernel.last_exec_time_ns = res.exec_time_ns
        kernel.last_results = res
    return out


# revision 33
# speedup vs baseline: 1.1731x; 1.1731x over previous
"""Trainium2 Bass kernel for the MFA/MPPCA mixture log-likelihood problem.

Math: out[n,k] = PI[k] + logprob[n,k] with Sigma_k = A_k A_k^T + diag(D_k^2),
computed via Woodbury.  Everything involving only the small parameters
(MU, A, D, PI) is folded on the host into:

    out[n,k] = CONST[k] + x[n]·H[:,k] + (x[n]^2)·G[:,k] + sum_l (x[n]·Csc[:,k,l])^2

where (with iD = D^-2, B = iD*A, L = I + A^T B, iL = inv(L), R = chol(iL),
C0 = B R, e = R^T B^T MU):
    G   = -0.5 * iD^T                       (d, K)
    H   = (iD*MU)^T - C0 e                  (d, K)
    Csc = sqrt(0.5) * C0                    (d, K*l)
    CONST = PI - 0.5*(d log 2pi + logdet Sigma + MU^T iD MU) + 0.5 |e|^2

Device kernel (data-parallel over N on 8 cores), all matmuls fp8e4 with
DoubleRow perf mode (2 fp8 weights/cell, 256-deep contraction per matmul):

  - Csc path: x8 chunk-pairs stationary, interleaved Csc columns moving;
    640 projections per sample land in two PSUM banks; ScalarE squares
    them (one 4D-AP activation), VectorE group-sums 10 -> 1 into the
    fp16 out1 tile.
  - H and G paths both accumulate into a [k, n] PSUM tile (h8/g8
    chunk-pairs stationary - cheap 64-col weight loads - with x8 / x2h
    as the 512-wide moving streams).  ScalarE copies it out with
    CONST[k] folded in as a per-partition bias -> fp16 out2.
  - x^2 is sent from the host as fp8 (x2h); the exact-residual term was
    measured to cost more than its accuracy is worth at rel-tol 2e-2.
  - The host sums out1[n,k] + out2[k,n]^T (cheap numpy) - this removes
    the PE transpose and the vector final-add from the critical path.
"""
import math
import numpy as np
import ml_dtypes

N_TOTAL, K, D_FEAT, L_FAC = 131072, 64, 512, 10
N_CORES = 8
N_PER_CORE = N_TOTAL // N_CORES  # 16384

CSC_COLS = K * L_FAC             # 640
NG_HALF = K // 2                 # 32 groups per psum bank
CSC_HALF = NG_HALF * L_FAC       # 320
GROUP = 1024                     # samples per DMA group (8 subtiles)

FP8 = ml_dtypes.float8_e4m3      # TRN fp8e4 (max +-240)


def host_prep(MU, A, D, PI):
    """Fold small-parameter math into matmul weights (float64 internally)."""
    MU64, A64, D64, PI64 = [np.asarray(v, np.float64) for v in (MU, A, D, PI)]
    Kc, d, l = A64.shape
    iD = D64 ** -2.0
    B = iD[..., None] * A64
    L = np.eye(l)[None] + np.einsum('kdl,kdm->klm', A64, B)
    sign, logdet_L = np.linalg.slogdet(L)
    log_det_Sigma = logdet_L - np.sum(np.log(iD), axis=1)
    iL = np.linalg.inv(L)
    R = np.linalg.cholesky(iL)                  # R @ R.T = iL
    C0 = np.einsum('kdl,klm->kdm', B, R)        # (K, d, l)
    bmu = np.einsum('kdl,kd->kl', B, MU64)
    e = np.einsum('klm,kl->km', R, bmu)         # (K, l)
    c1 = np.sum(iD * MU64 * MU64, axis=1)

    CONST = PI64 - 0.5 * (d * math.log(2.0 * math.pi) + log_det_Sigma + c1) \
        + 0.5 * np.sum(e * e, axis=1)
    G = (-0.5 * iD).T                                               # (d, K)
    H = (iD * MU64 - np.einsum('kdm,km->kd', C0, e)).T              # (d, K)
    Csc = (C0 * np.sqrt(0.5)).transpose(1, 0, 2).reshape(d, Kc * l)  # k-major

    # interleave chunk pairs: [p, pair, col, 2] so the DoubleRow moving
    # stream fetches both pair values from one 16B SBUF line
    wall_i = np.ascontiguousarray(
        Csc.astype(FP8).reshape(2, 2, 128, CSC_COLS).transpose(2, 0, 3, 1))
    hg8 = np.concatenate([H, G], axis=1).astype(FP8)                # (d, 2K)
    cwide = np.tile(CONST.astype(np.float32).reshape(K, 1),
                    (1, 512)).astype(np.float32)                    # (K, 512)
    return wall_i, hg8, cwide


def build_nc(n_per_core=N_PER_CORE):
    """Build and compile the Bass module for one core (SPMD across 8)."""
    import concourse.bacc as bacc
    import concourse.tile as tile
    import concourse.mybir as mybir

    f32 = mybir.dt.float32
    f16 = mybir.dt.float16
    f8 = mybir.dt.float8e4
    DR = mybir.MatmulPerfMode.DoubleRow
    DRSI = mybir.MatmulPerfMode.DoubleRowSwInterleave
    n_groups = n_per_core // GROUP
    n_sub_g = GROUP // 128       # 8 subtiles per DMA group
    assert n_per_core % GROUP == 0

    nc = bacc.Bacc("TRN2", target_bir_lowering=False, debug=False,
                   enable_asserts=False, num_devices=N_CORES)
    x8_dram = nc.dram_tensor("x8", (128, 2, n_per_core, 2), f8,
                             kind="ExternalInput")
    x2h_dram = nc.dram_tensor("x2h", (128, 2, n_per_core, 2), f8,
                              kind="ExternalInput")
    wall_dram = nc.dram_tensor("wall8i", (128, 2, CSC_COLS, 2), f8,
                               kind="ExternalInput")
    hg_dram = nc.dram_tensor("hg8", (D_FEAT, 2 * K), f8, kind="ExternalInput")
    cw_dram = nc.dram_tensor("cwide", (K, 512), f32, kind="ExternalInput")
    out1_dram = nc.dram_tensor("out1", (n_per_core, K), f16, kind="ExternalOutput")
    out2_dram = nc.dram_tensor("out2", (K, n_per_core), f16, kind="ExternalOutput")

    x8_v = x8_dram.ap()                                    # [128,2,n,2]
    x2h_v = x2h_dram.ap()                                  # [128,2,n,2]
    wall_v = wall_dram.ap()                                # [128,2,640,2]
    hg_v = hg_dram.ap().rearrange("(c p) m -> p c m", p=128)       # [128,4,128]

    with tile.TileContext(nc) as tc:
        with (
            tc.tile_pool(name="wpool", bufs=1) as wpool,
            tc.tile_pool(name="xpool", bufs=6) as xpool,
            tc.tile_pool(name="spool", bufs=6) as spool,
            tc.tile_pool(name="opool", bufs=4) as opool,
            tc.tile_pool(name="ppool", bufs=3, space="PSUM") as ppool,
            tc.tile_pool(name="gpool", bufs=1, space="PSUM") as gpool,
        ):
            # startup ordering: a 2-subtile head slice of x8 and the wall
            # go first so the first matmuls fire as early as possible; the
            # small weight tensors follow the group-0 bulk inputs
            x8h_sb = wpool.tile([128, 2, 256, 2], f8)
            nc.sync.dma_start(out=x8h_sb[:], in_=x8_v[:, :, 0:256, :])
            wall_sb = wpool.tile([128, 2, CSC_COLS, 2], f8)
            nc.sync.dma_start(out=wall_sb[:], in_=wall_v[:])

            def issue_in_dma(gi):
                gsl = slice(gi * GROUP, (gi + 1) * GROUP)
                x8_sb = xpool.tile([128, 2, GROUP, 2], f8, tag="x8")
                nc.sync.dma_start(out=x8_sb[:], in_=x8_v[:, :, gsl, :])
                x2h_sb = xpool.tile([128, 2, GROUP, 2], f8, tag="x2h")
                nc.sync.dma_start(out=x2h_sb[:], in_=x2h_v[:, :, gsl, :])
                return x8_sb, x2h_sb

            in_bufs = {0: issue_in_dma(0)}
            hg_sb = wpool.tile([128, 4, 2 * K], f8)
            nc.sync.dma_start(out=hg_sb[:], in_=hg_v[:])
            cw_sb = wpool.tile([K, 512], f32)
            nc.sync.dma_start(out=cw_sb[:], in_=cw_dram.ap())
            c_sb = cw_sb[:, 0:1]
            if n_groups > 1:
                in_bufs[1] = issue_in_dma(1)
            # deferred half-copies of ps_g -> out2 from the PREVIOUS group,
            # flushed early in the next group so they never park the scalar/
            # vector queues behind this group's compute
            deferred = []

            for gi in range(n_groups):
                gsl = slice(gi * GROUP, (gi + 1) * GROUP)
                # prefetch next group's inputs ahead of this group's output
                # DMAs so the sync queue never parks input transfers behind
                # an output waiting on compute
                if gi + 2 < n_groups:
                    in_bufs[gi + 2] = issue_in_dma(gi + 2)
                x8_sb, x2h_sb = in_bufs.pop(gi)

                out_t = opool.tile([128, n_sub_g, K], f16, tag="out")
                out2_t = opool.tile([K, GROUP], f16, tag="out2")
                ps_g0 = gpool.tile([K, 512], f32, tag="psg0")
                ps_g1 = gpool.tile([K, 512], f32, tag="psg1")
                halves = [ps_g0, ps_g1]

                # one H/G matmul per subtile, spread across the group so the
                # PE fills the scalar-paced slack instead of a serial tail
                def psg_mm(j):
                    h, idx = j // 4, j % 4
                    hsl = slice(h * 512, (h + 1) * 512)
                    dst = halves[h][:]
                    if idx < 2:
                        cpair, csl = idx * 2, slice(0, K)
                        rhs = x8_sb[:, idx, hsl, :] \
                            .rearrange("p t two -> p two t")
                    else:
                        cpair, csl = (idx - 2) * 2, slice(K, 2 * K)
                        rhs = x2h_sb[:, (idx - 2), hsl, :] \
                            .rearrange("p n two -> p two n")
                    nc.tensor.matmul(
                        dst, hg_sb[:, cpair:cpair + 2, csl], rhs,
                        start=(idx == 0), stop=(idx == 3), perf_mode=DR,
                        skip_group_check=True)

                psg_delay = 2 if gi == 0 else 0
                for j in range(n_sub_g):
                    nsl = slice(j * 128, (j + 1) * 128)
                    # psum: bank0 = [Csc groups 0:32], bank1 = [groups 32:64]
                    ps = ppool.tile([128, 1024], f32, tag="ps")
                    stat_sb = x8h_sb if (gi == 0 and j < 2) else x8_sb

                    def dr(dst, cpair, cols, start, stop):
                        # stationary: interleaved+reversed x pairs (SwInterleave)
                        nc.tensor.matmul(
                            dst,
                            stat_sb[:, cpair // 2, nsl, :]
                            .rearrange("p t two -> p (t two)"),
                            wall_sb[:, cpair // 2, cols, :]
                            .rearrange("p n two -> p two n"),
                            start=start, stop=stop, perf_mode=DRSI,
                            skip_group_check=True)

                    if j - psg_delay >= 0:
                        psg_mm(j - psg_delay)
                    dr(ps[:, 0:CSC_HALF], 0, slice(0, CSC_HALF), True, False)
                    dr(ps[:, 512:512 + CSC_HALF], 0,
                       slice(CSC_HALF, CSC_COLS), True, False)
                    dr(ps[:, 0:CSC_HALF], 2, slice(0, CSC_HALF), False, True)
                    dr(ps[:, 512:512 + CSC_HALF], 2,
                       slice(CSC_HALF, CSC_COLS), False, True)

                    # squares of the 640 factor projections (flat 3D AP;
                    # the reduce re-views the same bytes as [p, 64, 10])
                    sq = spool.tile([128, 2, CSC_HALF], f16, tag="sq")
                    nc.scalar.square(
                        sq[:],
                        ps.rearrange("p (b h) -> p b h", b=2)[:, :, 0:CSC_HALF])

                    # group-of-10 sums straight into the fp16 output tile
                    with nc.allow_low_precision("output is fp16 anyway"):
                        nc.vector.reduce_sum(
                            out_t[:, j, :],
                            sq.rearrange("p b (g t) -> p (b g) t", t=L_FAC),
                            axis=mybir.AxisListType.X)

                    # flush one deferred op from the previous group per
                    # subtile (keeps them off the critical path)
                    if deferred:
                        deferred.pop(0)()

                    if j == 5:
                        # ps_g half 0 complete: copy with CONST bias (scalar)
                        def copy_h0(t2=out2_t, pg=ps_g0):
                            nc.scalar.add(t2[:, 0:512], pg[:], add=c_sb[:])
                        deferred.append(copy_h0)

                for jj in range(n_sub_g - psg_delay, n_sub_g):
                    psg_mm(jj)

                # half 1 + out DMAs, deferred into the next group's subtiles
                def copy_h1(t2=out2_t, pg=ps_g1):
                    # vector does half 1: psum + CONST_wide -> fp16
                    nc.vector.tensor_add(t2[:, 512:1024], pg[:], cw_sb[:])
                deferred.append(copy_h1)

                def dma_out(ot=out_t, t2=out2_t, osl=gsl):
                    nc.sync.dma_start(
                        out=out1_dram.ap()[osl, :].rearrange(
                            "(j p) k -> p j k", p=128),
                        in_=ot[:])
                    nc.sync.dma_start(out=out2_dram.ap()[:, osl], in_=t2[:])
                deferred.append(dma_out)

            for f in deferred:
                f()

    nc.compile()
    return nc


_NC_CACHE = {}


def _get_nc(n_per_core=N_PER_CORE):
    if n_per_core not in _NC_CACHE:
        _NC_CACHE[n_per_core] = build_nc(n_per_core)
    return _NC_CACHE[n_per_core]


def _install_ntff_hook():
    """Provide the antenv.axon_hooks shim so trace=True can capture NTFFs."""
    import sys
    if "antenv.axon_hooks" in sys.modules:
        return
    import types
    import ctypes
    import contextlib

    so_path = "/opt/axon/libaxon_pjrt.so"
    lib = ctypes.CDLL(so_path)
    if not hasattr(lib, "axon_start_nrt_profile"):
        return
    lib.axon_start_nrt_profile.argtypes = [ctypes.POINTER(ctypes.c_int64), ctypes.c_size_t]
    lib.axon_start_nrt_profile.restype = ctypes.c_int64
    lib.axon_stop_nrt_profile.argtypes = [ctypes.c_char_p]
    lib.axon_stop_nrt_profile.restype = ctypes.c_int64

    @contextlib.contextmanager
    def _hook(output_dir, device_ids):
        import jax
        jax.devices()
        if device_ids:
            ids = (ctypes.c_int64 * len(device_ids))(*device_ids)
            rc = lib.axon_start_nrt_profile(ids, len(device_ids))
        else:
            rc = lib.axon_start_nrt_profile(None, 0)
        if rc != 0:
            raise RuntimeError(f"axon_start_nrt_profile rc={rc}")
        try:
            yield
        finally:
            n = lib.axon_stop_nrt_profile(str(output_dir).encode())
            print(f"ntff profile: {n} file(s) written to {output_dir}")

    mod = types.ModuleType("antenv.axon_hooks")
    mod.get_axon_ntff_profile_hook = lambda: _hook
    mod.set_axon_ntff_profile_hook = lambda h: None
    sys.modules["antenv.axon_hooks"] = mod


def kernel(x, MU, A, D, PI, trace=False):
    from concourse.bass_utils import run_bass_kernel_spmd
    if trace:
        try:
            _install_ntff_hook()
        except Exception as e:
            print(f"ntff hook install failed: {e}")
            trace = False

    x = np.asarray(x, np.float32)
    wall, hg8, cwide = host_prep(MU, A, D, PI)
    nc = _get_nc()

    def ileave_rev(a):
        # (512, n) -> (128, pair, n, 2): chunk-pair interleave with samples
        # reversed within each 128-block (DoubleRowSwInterleave weight order;
        # the H/G moving streams use the same order so psum columns agree)
        n = a.shape[1]
        return np.ascontiguousarray(
            a.reshape(2, 2, 128, n // 128, 128)[..., ::-1]
            .transpose(2, 0, 3, 4, 1).reshape(128, 2, n, 2))

    in_maps = []
    for c in range(N_CORES):
        xs = np.ascontiguousarray(x[c * N_PER_CORE:(c + 1) * N_PER_CORE, :].T)
        x8 = xs.astype(FP8)
        x2h = (xs * xs).astype(FP8)
        in_maps.append({"x8": ileave_rev(x8), "x2h": ileave_rev(x2h),
                        "wall8i": wall, "hg8": hg8, "cwide": cwide})

    res = run_bass_kernel_spmd(nc, in_maps, list(range(N_CORES)), trace=trace)
    parts = []
    for c in range(N_CORES):
        o1 = res.results[c]["out1"].astype(np.float32)
        o2 = res.results[c]["out2"].astype(np.float32)
        o2 = o2.reshape(K, -1, 128)[:, :, ::-1].reshape(K, N_PER_CORE)
        parts.append(o1 + o2.T)
    out = np.concatenate(parts, axis=0)
    if trace:
        kernel.last_exec_time_ns = res.exec_time_ns
        kernel.last_results = res
    return out
